# revision 72
# baseline (speedup 1.0000x reference)
"""Trainium2 Bass kernel for GQA attention (B=2, S=2048, HID=2048, H=16, G=4, D=128).

Sharding: 8 cores = 2 batches x 4 kv-groups. Core c handles batch c//4 and
kv-group c%4 (4 q heads + 1 kv head). Wq/Wk/Wv column-sharded by head group,
Wo row-sharded; per-core partial outputs are summed on the host per batch
(the unshard / all-reduce step).

v2: fp16 operands everywhere (PE streams 1 cyc/row same as fp32r>=256 but
halves DMA and unlocks DVE 16-bit modes), folded-sign sin table (4 DVE ops
per RoPE tensor instead of 6), single interleaved loop so projections of
chunk c+1 overlap attention of chunk c, minimal startup DMA.
"""

import os
import sys

sys.path.insert(0, "/opt/trn_rl_repo")

import numpy as np

B, S, HID = 2, 2048, 2048
H, G, D = 16, 4, 128
REP = H // G  # q heads per kv head = 4
NEG = -1e30
SCALE = 1.0 / np.sqrt(np.float32(D))

NKT = HID // 128  # 16 contraction tiles for projections
NSC = S // 512  # 4 s-chunks of 512
NST = S // 128  # 16 s-tiles of 128


def _emit(nc, tc, aps):
    """Emit the per-core program. aps: dict of DRAM APs."""
    from contextlib import ExitStack

    from concourse import mybir

    f16 = mybir.dt.float16
    f32 = mybir.dt.float32
    Exp = mybir.ActivationFunctionType.Exp
    Copy = mybir.ActivationFunctionType.Copy

    xT, cosT, sinT = aps["xT"], aps["cosT"], aps["sinT"]
    wq, wk, wv, wo = aps["wq"], aps["wk"], aps["wv"], aps["wo"]
    mask, ident, ones = aps["mask"], aps["ident"], aps["ones"]
    out = aps["out"]

    # DRAM views with 128-partition tiling of the contraction dim
    xT_t = xT.rearrange("(t p) s -> p t s", p=128)  # [128, 16, 2048]
    wq_t = wq.rearrange("(t p) m -> p t m", p=128)  # [128, 16, 512]
    wk_t = wk.rearrange("(t p) m -> p t m", p=128)  # [128, 16, 128]
    wv_t = wv.rearrange("(t p) m -> p t m", p=128)  # [128, 16, 128]
    wo_t = wo.rearrange("(t p) n -> p t n", p=128)  # [128, 4, 2048]

    lo = slice(0, 64)
    hi = slice(64, 128)

    with ExitStack() as ctx:
        persist = ctx.enter_context(tc.tile_pool(name="persist", bufs=1))
        xpool = ctx.enter_context(tc.tile_pool(name="xslab", bufs=2))
        rpool = ctx.enter_context(tc.tile_pool(name="rope", bufs=3))
        epool = ctx.enter_context(tc.tile_pool(name="eslab", bufs=6))
        rcpool = ctx.enter_context(tc.tile_pool(name="recip", bufs=3))
        ocpool = ctx.enter_context(tc.tile_pool(name="ocopy", bufs=4))
        vcpool = ctx.enter_context(tc.tile_pool(name="vcopy", bufs=3))
        # PSUM: acc 3 + sp/misc 3 + av/den 2 = 8 banks
        ppool = ctx.enter_context(tc.tile_pool(name="projps", bufs=3, space="PSUM"))
        spool = ctx.enter_context(tc.tile_pool(name="scps", bufs=3, space="PSUM"))
        apool = ctx.enter_context(tc.tile_pool(name="accps", bufs=2, space="PSUM"))

        # --- persistent tiles ---
        wq_sb = persist.tile([128, NKT, 512], f16, tag="wq", name="wq_sb")
        wk_sb = persist.tile([128, NKT, 128], f16, tag="wk", name="wk_sb")
        wv_sb = persist.tile([128, NKT, 128], f16, tag="wv", name="wv_sb")
        wo_sb = persist.tile([128, REP, HID], f16, tag="wo", name="wo_sb")
        cos_sb = persist.tile([128, S], f16, tag="cos", name="cos_sb")
        sin_sb = persist.tile([128, S], f16, tag="sin", name="sin_sb")
        mask_t = persist.tile([128, 128], f32, tag="mask", name="mask_t")
        ident_t = persist.tile([128, 128], f16, tag="ident", name="ident_t")
        ones_t = persist.tile([128, 128], f16, tag="ones", name="ones_t")
        qt = [
            persist.tile([128, S], f16, tag=f"qt{h}", name=f"qt{h}")
            for h in range(REP)
        ]
        kt_sb = persist.tile([128, S], f16, tag="kt", name="kt_sb")
        v_sb = persist.tile([128, NST, 128], f16, tag="v", name="v_sb")
        aot = persist.tile([128, REP, S], f16, tag="aot", name="aot")

        # --- startup DMA: interleave the first k-tiles of x and W so the
        # first matmul is gated on ~1.3MB, not the whole 9MB preload.
        # mask is the very first transfer (64KB on the sync DGE): it gates
        # the PE warm-up dummies, which should start as early as possible
        nc.sync.dma_start(mask_t[:], mask)
        nc.gpsimd.dma_start(ident_t[:], ident)
        nc.gpsimd.dma_start(ones_t[:], ones)
        # trig tables on the scalar DGE: rope(k) releases the PSUM slot that
        # gates the q-wave, so cos/sin must not queue behind the x stream
        nc.scalar.dma_start(cos_sb[:], cosT)
        nc.scalar.dma_start(sin_sb[:], sinT)

        # PE warm-up: the HAM clock gate holds the PE at 1.2 GHz until it has
        # seen ~3.4us of sustained activity. Burn dummy fp32 matmuls (slow on
        # purpose: 4 cyc/row each, lowered to 2 HW matmuls) on the mask tile
        # while the first x/w DMA pieces stream in, so the real projection
        # waves start at full clock. Count is tuned so the chain ends right
        # at the clock-gate flip — more just delays the queued real work.
        warm = spool.tile([128, 512], f32, tag="sp", name="warm")
        for _ in range(9):
            nc.tensor.matmul(
                warm[:, 0:128], lhsT=mask_t[:], rhs=mask_t[:], start=True, stop=True
            )

        xs0 = xpool.tile([128, NKT, 512], f16, tag="x", name="xs0")
        pieces = [slice(0, 2), slice(2, 4), slice(4, 8), slice(8, 12), slice(12, 16)]
        # x/wk/wv pieces first — they gate the leading (k,v) wave; wq pieces
        # follow two pieces behind (q waves start ~7us later)
        for i, ps in enumerate(pieces):
            nc.sync.dma_start(xs0[:, ps, :], xT_t[:, ps, 0:512])
            nc.sync.dma_start(wk_sb[:, ps, :], wk_t[:, ps, :])
            nc.sync.dma_start(wv_sb[:, ps, :], wv_t[:, ps, :])
            if i >= 2:
                nc.sync.dma_start(wq_sb[:, pieces[i - 2], :], wq_t[:, pieces[i - 2], :])
        for ps in pieces[3:]:
            nc.sync.dma_start(wq_sb[:, ps, :], wq_t[:, ps, :])

        def rope(acc, dest, cs):
            """dest[:, cs] = acc*cos + rot_half(acc)*sin  (sin sign-folded)."""
            tmp_a = rpool.tile([128, 512], f16, tag="tmpa", name="tmp_a")
            tmp_b = rpool.tile([128, 512], f16, tag="tmpb", name="tmp_b")
            nc.vector.tensor_mul(tmp_a[:], acc[:], cos_sb[:, cs])
            nc.vector.tensor_mul(tmp_b[lo, :], acc[hi, :], sin_sb[lo, cs])
            nc.vector.tensor_mul(tmp_b[hi, :], acc[lo, :], sin_sb[hi, cs])
            nc.vector.tensor_add(dest[:, cs], tmp_a[:], tmp_b[:])

        xs = xs0
        xs_next = None
        for c in range(NSC):
            cs = slice(c * 512, (c + 1) * 512)
            # ---------- prefetch next x slab, then projections for chunk c --
            if c > 0:
                xs = xs_next
            if c < NSC - 1:
                ns = slice((c + 1) * 512, (c + 2) * 512)
                xs_next = xpool.tile([128, NKT, 512], f16, tag="x", name="xs")
                for p4 in range(4):
                    p4s = slice(p4 * 4, (p4 + 1) * 4)
                    nc.sync.dma_start(xs_next[:, p4s, :], xT_t[:, p4s, ns])
            # projections in 3 waves of 2 PSUM accumulators: (k,v), (q0,q1),
            # (q2,q3) — with ppool at 3 there is always a spare slot so the
            # next wave starts before the previous one's RoPE drains
            for w in range(3):
                if w == 0:
                    k_acc = ppool.tile([128, 512], f32, tag="acc", name="k_acc")
                    v_acc = ppool.tile([128, 512], f32, tag="acc", name="v_acc")
                else:
                    hq = (2 * (w - 1), 2 * (w - 1) + 1)
                    q_acc = {
                        h: ppool.tile([128, 512], f32, tag="acc", name=f"q_acc{h}")
                        for h in hq
                    }
                for g in range(NKT):
                    st = g == 0
                    sp = g == NKT - 1
                    rhs = xs[:, g, :]
                    if w == 0:
                        nc.tensor.matmul(
                            k_acc[:], lhsT=wk_sb[:, g, :], rhs=rhs, start=st, stop=sp
                        )
                        nc.tensor.matmul(
                            v_acc[:], lhsT=wv_sb[:, g, :], rhs=rhs, start=st, stop=sp
                        )
                    else:
                        for h in hq:
                            nc.tensor.matmul(
                                q_acc[h][:],
                                lhsT=wq_sb[:, g, h * 128 : (h + 1) * 128],
                                rhs=rhs,
                                start=st,
                                stop=sp,
                            )
                if w == 0:
                    rope(k_acc, kt_sb, cs)
                    # V: copy chunk to fp16, then PE-transpose per s-tile
                    vt_c = vcpool.tile([128, 512], f16, tag="vt", name="vt_c")
                    nc.any.tensor_copy(vt_c[:], v_acc[:])
                    for j in range(4):
                        i = c * 4 + j
                        vps = spool.tile([128, 512], f16, tag="sp", name="vps")
                        nc.tensor.transpose(
                            vps[:, 0:128], vt_c[:, j * 128 : (j + 1) * 128], ident_t[:]
                        )
                        nc.vector.tensor_copy(v_sb[:, i, :], vps[:, 0:128])
                else:
                    for h in hq:
                        rope(q_acc[h], qt[h], cs)

            # ---------- output projection helper ----------
            def out_tile(st_i, hc):
                ss = slice(st_i * 128, (st_i + 1) * 128)
                hs = slice(hc * 512, (hc + 1) * 512)
                ops = spool.tile([128, 512], f32, tag="sp", name="ops")
                for m in range(REP):
                    nc.tensor.matmul(
                        ops[:],
                        lhsT=aot[:, m, ss],
                        rhs=wo_sb[:, m, hs],
                        start=(m == 0),
                        stop=(m == REP - 1),
                    )
                oc = ocpool.tile([128, 512], f16, tag="oc", name="oc")
                if st_i >= 8:
                    # deferred chunks run in the exp-bound tail: keep the
                    # PSUM->SBUF copies off the Scalar engine
                    nc.vector.tensor_copy(oc[:], ops[:])
                else:
                    # split halves across Scalar and Vector: the copy gates
                    # the ops PSUM slot release, and halves run in parallel
                    nc.scalar.activation(oc[:, 0:256], ops[:, 0:256], Copy)
                    nc.vector.tensor_copy(oc[:, 256:512], ops[:, 256:512])
                # gpsimd DGE: keeps the 64 out-DMA issues off the sync
                # sequencer's prefetch stream (gpsimd is otherwise idle)
                nc.gpsimd.dma_start(out[ss, hs], oc[:])



            # ---------- attention for chunk c ----------
            for h in range(REP):
                av = apool.tile([128, 512], f32, tag="av", name="av")
                # running sum of exp tiles on DVE; one ones-matmul per (h, c)
                # replaces per-tile denominator matmuls
                wsum = vcpool.tile([128, 512], f16, tag="ws", name="wsum")
                for ki, kb in enumerate(range(c + 1)):
                    es = epool.tile([128, 4, 512], f16, tag="es", name="es")
                    for j in range(4):
                        i = kb * 4 + j
                        sp_t = spool.tile([128, 512], f32, tag="sp", name="sp_t")
                        j0 = j * 128 if kb == c else 0
                        nc.tensor.matmul(
                            sp_t[:, j0:512],
                            lhsT=kt_sb[:, i * 128 : (i + 1) * 128],
                            rhs=qt[h][:, c * 512 + j0 : (c + 1) * 512],
                            start=True,
                            stop=True,
                        )
                        if kb == c:
                            # diagonal block: q sub-block j partially masked
                            nc.vector.tensor_add(
                                sp_t[:, j * 128 : (j + 1) * 128],
                                sp_t[:, j * 128 : (j + 1) * 128],
                                mask_t[:],
                            )
                        nc.scalar.activation(
                            es[:, j, j0:512],
                            sp_t[:, j0:512],
                            Exp,
                            scale=float(SCALE),
                        )
                    if kb == c:
                        # diagonal slab: es[j] only valid in [j*128, 512)
                        if c == 0:
                            nc.vector.tensor_copy(wsum[:], es[:, 0, :])
                        else:
                            nc.vector.tensor_add(wsum[:], wsum[:], es[:, 0, :])
                        for j in range(1, 4):
                            js = slice(j * 128, 512)
                            nc.vector.tensor_add(
                                wsum[:, js], wsum[:, js], es[:, j, js]
                            )
                    elif kb == 0:
                        nc.vector.tensor_add(wsum[:], es[:, 0, :], es[:, 1, :])
                        nc.vector.tensor_add(wsum[:], wsum[:], es[:, 2, :])
                        nc.vector.tensor_add(wsum[:], wsum[:], es[:, 3, :])
                    else:
                        for j in range(4):
                            nc.vector.tensor_add(wsum[:], wsum[:], es[:, j, :])
                    for j in range(4):
                        i = kb * 4 + j
                        st = i == 0
                        sp = i == 4 * c + 3
                        j0 = j * 128 if kb == c else 0
                        nc.tensor.matmul(
                            av[:, j0:512],
                            lhsT=v_sb[:, i, :],
                            rhs=es[:, j, j0:512],
                            start=st,
                            stop=sp,
                        )
                den = apool.tile([128, 512], f32, tag="av", name="den")
                nc.tensor.matmul(
                    den[:], lhsT=ones_t[:], rhs=wsum[:], start=True, stop=True
                )
                rc = rcpool.tile([128, 512], f32, tag="rc", name="rc")
                nc.vector.reciprocal_approx_fast(rc[:], den[:])
                nc.vector.tensor_mul(aot[:, h, cs], av[:], rc[:])

            if c == 0:
                for p4 in range(REP):
                    nc.sync.dma_start(wo_sb[:, p4, :], wo_t[:, p4, :])
            if c < 2:
                for st_i in range(4 * c, 4 * c + 4):
                    for hc in range(NSC):
                        out_tile(st_i, hc)
            elif c == 3:
                for st_i in range(8, 16):
                    for hc in range(NSC):
                        out_tile(st_i, hc)


def build_program():
    import concourse.tile as tile
    from concourse import bacc, mybir

    f16 = mybir.dt.float16
    f32 = mybir.dt.float32
    nc = bacc.Bacc("TRN2", target_bir_lowering=False, debug=False, num_devices=8)
    aps = {}
    aps["xT"] = nc.dram_tensor("xT", [HID, S], f16, kind="ExternalInput").ap()
    aps["cosT"] = nc.dram_tensor("cosT", [D, S], f16, kind="ExternalInput").ap()
    aps["sinT"] = nc.dram_tensor("sinT", [D, S], f16, kind="ExternalInput").ap()
    aps["wq"] = nc.dram_tensor("wq", [HID, REP * D], f16, kind="ExternalInput").ap()
    aps["wk"] = nc.dram_tensor("wk", [HID, D], f16, kind="ExternalInput").ap()
    aps["wv"] = nc.dram_tensor("wv", [HID, D], f16, kind="ExternalInput").ap()
    aps["wo"] = nc.dram_tensor("wo", [REP * D, HID], f16, kind="ExternalInput").ap()
    aps["mask"] = nc.dram_tensor("mask", [128, 128], f32, kind="ExternalInput").ap()
    aps["ones"] = nc.dram_tensor("ones", [128, 128], f16, kind="ExternalInput").ap()
    aps["ident"] = nc.dram_tensor("ident", [128, 128], f16, kind="ExternalInput").ap()
    aps["out"] = nc.dram_tensor("out", [S, HID], f16, kind="ExternalOutput").ap()

    with tile.TileContext(nc) as tc:
        _emit(nc, tc, aps)
    nc.compile()
    return nc


def make_in_maps(x, cos, sin, Wq, Wk, Wv, Wo):
    """Build the 8 per-core input dicts. Core c: batch c//4, kv-group c%4."""
    f16 = np.float16
    mask = np.where(
        np.arange(128)[:, None] <= np.arange(128)[None, :], 0.0, NEG
    ).astype(np.float32)
    ident = np.eye(128, dtype=f16)
    ones = np.ones((128, 128), dtype=f16)
    xT = [np.ascontiguousarray(x[b].T).astype(f16) for b in range(B)]
    cosT = np.ascontiguousarray(cos.T).astype(f16)
    # fold the rotate-half sign into the sin table: lo half negated
    sinT = np.ascontiguousarray(sin.T).astype(np.float32)
    sinT[:64] = -sinT[:64]
    sinT = sinT.astype(f16)
    in_maps = []
    for c in range(8):
        b, g = c // 4, c % 4
        in_maps.append(
            {
                "xT": xT[b],
                "cosT": cosT,
                "sinT": sinT,
                "wq": Wq[:, g * REP * D : (g + 1) * REP * D].astype(f16),
                "wk": Wk[:, g * D : (g + 1) * D].astype(f16),
                "wv": Wv[:, g * D : (g + 1) * D].astype(f16),
                "wo": Wo[g * REP * D : (g + 1) * REP * D, :].astype(f16),
                "mask": mask,
                "ident": ident,
                "ones": ones,
            }
        )
    return in_maps


def kernel(x, cos, sin, Wq, Wk, Wv, Wo):
    from concourse import bass_utils

    nc = build_program()
    in_maps = make_in_maps(x, cos, sin, Wq, Wk, Wv, Wo)
    trace = bool(int(os.environ.get("BASS_KERNEL_TRACE", "0")))
    res = bass_utils.run_bass_kernel_spmd(
        nc,
        in_maps,
        core_ids=list(range(8)),
        trace=trace,
    )
    if trace:
        print(f"HW exec time: {res.exec_time_ns} ns")
        if res.instructions_and_trace is not None:
            print(f"trace: {res.instructions_and_trace[1]}")
    out = np.empty((B, S, HID), dtype=np.float32)
    for b in range(B):
        acc = res.results[4 * b]["out"].astype(np.float32)
        for g in range(1, G):
            acc += res.results[4 * b + g]["out"].astype(np.float32)
        out[b] = acc
    return out


# revision 74
# speedup vs baseline: 1.0438x; 1.0438x over previous
"""Trainium2 Bass kernel for GQA attention (B=2, S=2048, HID=2048, H=16, G=4, D=128).

Sharding: 8 cores = 2 batches x 4 kv-groups. Core c handles batch c//4 and
kv-group c%4 (4 q heads + 1 kv head). Wq/Wk/Wv column-sharded by head group,
Wo row-sharded; per-core partial outputs are summed on the host per batch
(the unshard / all-reduce step).

v2: fp16 operands everywhere (PE streams 1 cyc/row same as fp32r>=256 but
halves DMA and unlocks DVE 16-bit modes), folded-sign sin table (4 DVE ops
per RoPE tensor instead of 6), single interleaved loop so projections of
chunk c+1 overlap attention of chunk c, minimal startup DMA.
"""

import os
import sys

sys.path.insert(0, "/opt/trn_rl_repo")

import numpy as np

B, S, HID = 2, 2048, 2048
H, G, D = 16, 4, 128
REP = H // G  # q heads per kv head = 4
NEG = -1e30
SCALE = 1.0 / np.sqrt(np.float32(D))

NKT = HID // 128  # 16 contraction tiles for projections
NSC = S // 512  # 4 s-chunks of 512
NST = S // 128  # 16 s-tiles of 128


def _emit(nc, tc, aps):
    """Emit the per-core program. aps: dict of DRAM APs."""
    from contextlib import ExitStack

    from concourse import mybir

    f16 = mybir.dt.float16
    f32 = mybir.dt.float32
    Exp = mybir.ActivationFunctionType.Exp

    xT, cosT, sinT = aps["xT"], aps["cosT"], aps["sinT"]
    wq, wk, wv, wo = aps["wq"], aps["wk"], aps["wv"], aps["wo"]
    mask, ident, ones = aps["mask"], aps["ident"], aps["ones"]
    out = aps["out"]

    # DRAM views with 128-partition tiling of the contraction dim
    xT_t = xT.rearrange("(t p) s -> p t s", p=128)  # [128, 16, 2048]
    wq_t = wq.rearrange("(t p) m -> p t m", p=128)  # [128, 16, 512]
    wk_t = wk.rearrange("(t p) m -> p t m", p=128)  # [128, 16, 128]
    wv_t = wv.rearrange("(t p) m -> p t m", p=128)  # [128, 16, 128]
    wo_t = wo.rearrange("(t p) n -> p t n", p=128)  # [128, 4, 2048]

    lo = slice(0, 64)
    hi = slice(64, 128)

    with ExitStack() as ctx:
        persist = ctx.enter_context(tc.tile_pool(name="persist", bufs=1))
        xpool = ctx.enter_context(tc.tile_pool(name="xslab", bufs=2))
        rpool = ctx.enter_context(tc.tile_pool(name="rope", bufs=3))
        epool = ctx.enter_context(tc.tile_pool(name="eslab", bufs=6))
        rcpool = ctx.enter_context(tc.tile_pool(name="recip", bufs=3))
        ocpool = ctx.enter_context(tc.tile_pool(name="ocopy", bufs=4))
        vcpool = ctx.enter_context(tc.tile_pool(name="vcopy", bufs=3))
        # PSUM: acc 3 + sp/misc 3 + av/den 2 = 8 banks
        ppool = ctx.enter_context(tc.tile_pool(name="projps", bufs=3, space="PSUM"))
        spool = ctx.enter_context(tc.tile_pool(name="scps", bufs=3, space="PSUM"))
        apool = ctx.enter_context(tc.tile_pool(name="accps", bufs=2, space="PSUM"))

        # --- persistent tiles ---
        wq_sb = persist.tile([128, NKT, 512], f16, tag="wq", name="wq_sb")
        wk_sb = persist.tile([128, NKT, 128], f16, tag="wk", name="wk_sb")
        wv_sb = persist.tile([128, NKT, 128], f16, tag="wv", name="wv_sb")
        wo_sb = persist.tile([128, REP, HID], f16, tag="wo", name="wo_sb")
        cos_sb = persist.tile([128, S], f16, tag="cos", name="cos_sb")
        sin_sb = persist.tile([128, S], f16, tag="sin", name="sin_sb")
        mask_t = persist.tile([128, 128], f32, tag="mask", name="mask_t")
        ident_t = persist.tile([128, 128], f16, tag="ident", name="ident_t")
        ones_t = persist.tile([128, 128], f16, tag="ones", name="ones_t")
        qt = [
            persist.tile([128, S], f16, tag=f"qt{h}", name=f"qt{h}")
            for h in range(REP)
        ]
        kt_sb = persist.tile([128, S], f16, tag="kt", name="kt_sb")
        v_sb = persist.tile([128, NST, 128], f16, tag="v", name="v_sb")
        aot = persist.tile([128, REP, S], f16, tag="aot", name="aot")

        # --- startup DMA: interleave the first k-tiles of x and W so the
        # first matmul is gated on ~1.3MB, not the whole 9MB preload.
        # mask is the very first transfer (64KB on the sync DGE): it gates
        # the PE warm-up dummies, which should start as early as possible
        nc.sync.dma_start(mask_t[:], mask)
        nc.gpsimd.dma_start(ident_t[:], ident)
        nc.gpsimd.dma_start(ones_t[:], ones)
        # trig tables on the scalar DGE: rope(k) releases the PSUM slot that
        # gates the q-wave, so cos/sin must not queue behind the x stream
        nc.scalar.dma_start(cos_sb[:], cosT)
        nc.scalar.dma_start(sin_sb[:], sinT)

        # PE warm-up: the HAM clock gate holds the PE at 1.2 GHz until it has
        # seen ~3.4us of sustained activity. Burn dummy fp32 matmuls (slow on
        # purpose: 4 cyc/row each, lowered to 2 HW matmuls) on the mask tile
        # while the first x/w DMA pieces stream in, so the real projection
        # waves start at full clock. Count is tuned so the chain ends right
        # at the clock-gate flip — more just delays the queued real work.
        warm = spool.tile([128, 512], f32, tag="sp", name="warm")
        for _ in range(9):
            nc.tensor.matmul(
                warm[:, 0:128], lhsT=mask_t[:], rhs=mask_t[:], start=True, stop=True
            )

        xs0 = xpool.tile([128, NKT, 512], f16, tag="x", name="xs0")
        pieces = [slice(0, 2), slice(2, 4), slice(4, 8), slice(8, 12), slice(12, 16)]
        # x/wk/wv pieces first — they gate the leading (k,v) wave; wq pieces
        # follow two pieces behind (q waves start ~7us later)
        for i, ps in enumerate(pieces):
            nc.sync.dma_start(xs0[:, ps, :], xT_t[:, ps, 0:512])
            nc.sync.dma_start(wk_sb[:, ps, :], wk_t[:, ps, :])
            nc.sync.dma_start(wv_sb[:, ps, :], wv_t[:, ps, :])
            if i >= 2:
                nc.sync.dma_start(wq_sb[:, pieces[i - 2], :], wq_t[:, pieces[i - 2], :])
        for ps in pieces[3:]:
            nc.sync.dma_start(wq_sb[:, ps, :], wq_t[:, ps, :])

        def rope(acc, dest, cs):
            """dest[:, cs] = acc*cos + rot_half(acc)*sin  (sin sign-folded)."""
            tmp_a = rpool.tile([128, 512], f16, tag="tmpa", name="tmp_a")
            tmp_b = rpool.tile([128, 512], f16, tag="tmpb", name="tmp_b")
            nc.vector.tensor_mul(tmp_a[:], acc[:], cos_sb[:, cs])
            nc.vector.tensor_mul(tmp_b[lo, :], acc[hi, :], sin_sb[lo, cs])
            nc.vector.tensor_mul(tmp_b[hi, :], acc[lo, :], sin_sb[hi, cs])
            nc.vector.tensor_add(dest[:, cs], tmp_a[:], tmp_b[:])

        xs = xs0
        xs_next = None
        for c in range(NSC):
            cs = slice(c * 512, (c + 1) * 512)
            # ---------- prefetch next x slab, then projections for chunk c --
            if c > 0:
                xs = xs_next
            if c < NSC - 1:
                ns = slice((c + 1) * 512, (c + 2) * 512)
                xs_next = xpool.tile([128, NKT, 512], f16, tag="x", name="xs")
                for p4 in range(4):
                    p4s = slice(p4 * 4, (p4 + 1) * 4)
                    nc.sync.dma_start(xs_next[:, p4s, :], xT_t[:, p4s, ns])
            # projections in 3 waves of 2 PSUM accumulators: (k,v), (q0,q1),
            # (q2,q3) — with ppool at 3 there is always a spare slot so the
            # next wave starts before the previous one's RoPE drains
            for w in range(3):
                if w == 0:
                    k_acc = ppool.tile([128, 512], f32, tag="acc", name="k_acc")
                    v_acc = ppool.tile([128, 512], f32, tag="acc", name="v_acc")
                else:
                    hq = (2 * (w - 1), 2 * (w - 1) + 1)
                    q_acc = {
                        h: ppool.tile([128, 512], f32, tag="acc", name=f"q_acc{h}")
                        for h in hq
                    }
                for g in range(NKT):
                    st = g == 0
                    sp = g == NKT - 1
                    rhs = xs[:, g, :]
                    if w == 0:
                        nc.tensor.matmul(
                            k_acc[:], lhsT=wk_sb[:, g, :], rhs=rhs, start=st, stop=sp
                        )
                        nc.tensor.matmul(
                            v_acc[:], lhsT=wv_sb[:, g, :], rhs=rhs, start=st, stop=sp
                        )
                    else:
                        for h in hq:
                            nc.tensor.matmul(
                                q_acc[h][:],
                                lhsT=wq_sb[:, g, h * 128 : (h + 1) * 128],
                                rhs=rhs,
                                start=st,
                                stop=sp,
                            )
                if w == 0:
                    rope(k_acc, kt_sb, cs)
                    # V: copy chunk to fp16, then PE-transpose per s-tile
                    vt_c = vcpool.tile([128, 512], f16, tag="vt", name="vt_c")
                    nc.any.tensor_copy(vt_c[:], v_acc[:])
                    for j in range(4):
                        i = c * 4 + j
                        vps = spool.tile([128, 512], f16, tag="sp", name="vps")
                        nc.tensor.transpose(
                            vps[:, 0:128], vt_c[:, j * 128 : (j + 1) * 128], ident_t[:]
                        )
                        nc.vector.tensor_copy(v_sb[:, i, :], vps[:, 0:128])
                else:
                    for h in hq:
                        rope(q_acc[h], qt[h], cs)

            # ---------- output projection helper ----------
            def out_tile(st_i, hc):
                ss = slice(st_i * 128, (st_i + 1) * 128)
                hs = slice(hc * 512, (hc + 1) * 512)
                ops = spool.tile([128, 512], f32, tag="sp", name="ops")
                for m in range(REP):
                    nc.tensor.matmul(
                        ops[:],
                        lhsT=aot[:, m, ss],
                        rhs=wo_sb[:, m, hs],
                        start=(m == 0),
                        stop=(m == REP - 1),
                    )
                oc = ocpool.tile([128, 512], f16, tag="oc", name="oc")
                if st_i >= 8:
                    # deferred chunks run in the exp-bound tail: keep the
                    # PSUM->SBUF copies off the Scalar engine
                    nc.vector.tensor_copy(oc[:], ops[:])
                else:
                    nc.any.tensor_copy(oc[:], ops[:])
                nc.sync.dma_start(out[ss, hs], oc[:])



            # ---------- attention for chunk c ----------
            for h in range(REP):
                av = apool.tile([128, 512], f32, tag="av", name="av")
                # running sum of exp tiles on DVE; one ones-matmul per (h, c)
                # replaces per-tile denominator matmuls
                wsum = vcpool.tile([128, 512], f16, tag="ws", name="wsum")
                for ki, kb in enumerate(range(c + 1)):
                    es = epool.tile([128, 4, 512], f16, tag="es", name="es")
                    for j in range(4):
                        i = kb * 4 + j
                        sp_t = spool.tile([128, 512], f32, tag="sp", name="sp_t")
                        j0 = j * 128 if kb == c else 0
                        nc.tensor.matmul(
                            sp_t[:, j0:512],
                            lhsT=kt_sb[:, i * 128 : (i + 1) * 128],
                            rhs=qt[h][:, c * 512 + j0 : (c + 1) * 512],
                            start=True,
                            stop=True,
                        )
                        if kb == c:
                            # diagonal block: q sub-block j partially masked
                            nc.vector.tensor_add(
                                sp_t[:, j * 128 : (j + 1) * 128],
                                sp_t[:, j * 128 : (j + 1) * 128],
                                mask_t[:],
                            )
                        nc.scalar.activation(
                            es[:, j, j0:512],
                            sp_t[:, j0:512],
                            Exp,
                            scale=float(SCALE),
                        )
                    if kb == c:
                        # diagonal slab: es[j] only valid in [j*128, 512)
                        if c == 0:
                            nc.vector.tensor_copy(wsum[:], es[:, 0, :])
                        else:
                            nc.vector.tensor_add(wsum[:], wsum[:], es[:, 0, :])
                        for j in range(1, 4):
                            js = slice(j * 128, 512)
                            nc.vector.tensor_add(
                                wsum[:, js], wsum[:, js], es[:, j, js]
                            )
                    elif kb == 0:
                        nc.vector.tensor_add(wsum[:], es[:, 0, :], es[:, 1, :])
                        nc.vector.tensor_add(wsum[:], wsum[:], es[:, 2, :])
                        nc.vector.tensor_add(wsum[:], wsum[:], es[:, 3, :])
                    else:
                        for j in range(4):
                            nc.vector.tensor_add(wsum[:], wsum[:], es[:, j, :])
                    for j in range(4):
                        i = kb * 4 + j
                        st = i == 0
                        sp = i == 4 * c + 3
                        j0 = j * 128 if kb == c else 0
                        nc.tensor.matmul(
                            av[:, j0:512],
                            lhsT=v_sb[:, i, :],
                            rhs=es[:, j, j0:512],
                            start=st,
                            stop=sp,
                        )
                den = apool.tile([128, 512], f32, tag="av", name="den")
                nc.tensor.matmul(
                    den[:], lhsT=ones_t[:], rhs=wsum[:], start=True, stop=True
                )
                rc = rcpool.tile([128, 512], f32, tag="rc", name="rc")
                nc.vector.reciprocal_approx_fast(rc[:], den[:])
                nc.vector.tensor_mul(aot[:, h, cs], av[:], rc[:])

            if c == 0:
                for p4 in range(REP):
                    nc.sync.dma_start(wo_sb[:, p4, :], wo_t[:, p4, :])
            if c < 2:
                for st_i in range(4 * c, 4 * c + 4):
                    for hc in range(NSC):
                        out_tile(st_i, hc)
            elif c == 3:
                for st_i in range(8, 16):
                    for hc in range(NSC):
                        out_tile(st_i, hc)


def build_program():
    import concourse.tile as tile
    from concourse import bacc, mybir

    f16 = mybir.dt.float16
    f32 = mybir.dt.float32
    nc = bacc.Bacc("TRN2", target_bir_lowering=False, debug=False, num_devices=8)
    aps = {}
    aps["xT"] = nc.dram_tensor("xT", [HID, S], f16, kind="ExternalInput").ap()
    aps["cosT"] = nc.dram_tensor("cosT", [D, S], f16, kind="ExternalInput").ap()
    aps["sinT"] = nc.dram_tensor("sinT", [D, S], f16, kind="ExternalInput").ap()
    aps["wq"] = nc.dram_tensor("wq", [HID, REP * D], f16, kind="ExternalInput").ap()
    aps["wk"] = nc.dram_tensor("wk", [HID, D], f16, kind="ExternalInput").ap()
    aps["wv"] = nc.dram_tensor("wv", [HID, D], f16, kind="ExternalInput").ap()
    aps["wo"] = nc.dram_tensor("wo", [REP * D, HID], f16, kind="ExternalInput").ap()
    aps["mask"] = nc.dram_tensor("mask", [128, 128], f32, kind="ExternalInput").ap()
    aps["ones"] = nc.dram_tensor("ones", [128, 128], f16, kind="ExternalInput").ap()
    aps["ident"] = nc.dram_tensor("ident", [128, 128], f16, kind="ExternalInput").ap()
    aps["out"] = nc.dram_tensor("out", [S, HID], f16, kind="ExternalOutput").ap()

    with tile.TileContext(nc) as tc:
        _emit(nc, tc, aps)
    nc.compile()
    return nc


def make_in_maps(x, cos, sin, Wq, Wk, Wv, Wo):
    """Build the 8 per-core input dicts. Core c: batch c//4, kv-group c%4."""
    f16 = np.float16
    mask = np.where(
        np.arange(128)[:, None] <= np.arange(128)[None, :], 0.0, NEG
    ).astype(np.float32)
    ident = np.eye(128, dtype=f16)
    ones = np.ones((128, 128), dtype=f16)
    xT = [np.ascontiguousarray(x[b].T).astype(f16) for b in range(B)]
    cosT = np.ascontiguousarray(cos.T).astype(f16)
    # fold the rotate-half sign into the sin table: lo half negated
    sinT = np.ascontiguousarray(sin.T).astype(np.float32)
    sinT[:64] = -sinT[:64]
    sinT = sinT.astype(f16)
    in_maps = []
    for c in range(8):
        b, g = c // 4, c % 4
        in_maps.append(
            {
                "xT": xT[b],
                "cosT": cosT,
                "sinT": sinT,
                "wq": Wq[:, g * REP * D : (g + 1) * REP * D].astype(f16),
                "wk": Wk[:, g * D : (g + 1) * D].astype(f16),
                "wv": Wv[:, g * D : (g + 1) * D].astype(f16),
                "wo": Wo[g * REP * D : (g + 1) * REP * D, :].astype(f16),
                "mask": mask,
                "ident": ident,
                "ones": ones,
            }
        )
    return in_maps


def kernel(x, cos, sin, Wq, Wk, Wv, Wo):
    from concourse import bass_utils

    nc = build_program()
    in_maps = make_in_maps(x, cos, sin, Wq, Wk, Wv, Wo)
    trace = bool(int(os.environ.get("BASS_KERNEL_TRACE", "0")))
    res = bass_utils.run_bass_kernel_spmd(
        nc,
        in_maps,
        core_ids=list(range(8)),
        trace=trace,
    )
    if trace:
        print(f"HW exec time: {res.exec_time_ns} ns")
        if res.instructions_and_trace is not None:
            print(f"trace: {res.instructions_and_trace[1]}")
    out = np.empty((B, S, HID), dtype=np.float32)
    for b in range(B):
        acc = res.results[4 * b]["out"].astype(np.float32)
        for g in range(1, G):
            acc += res.results[4 * b + g]["out"].astype(np.float32)
        out[b] = acc
    return out
